# revision 14
# baseline (speedup 1.0000x reference)
"""Trainium2 Bass kernel for ConversationAwareRGCNLayer (8 NeuronCores).

Sharding: destination-sharded. Core c owns dst rows [c*D, (c+1)*D) for both
posts and users (D = 12512) and receives exactly the edges that point into
its slice, so per-core outputs are disjoint and no collectives are needed.

Math (linearity of segment-sum):
  post_pub = (seg_sum(h_user[pub_src]) @ W_pub + cnt*b_pub) / max(cnt,1)
  post_com = (0.7*seg_sum(h_user[com_src]) @ W_com
              + 0.3*seg_sum(e_comment) @ W_ecom
              + cnt*(0.7 b_com + 0.3 b_ecom)) / max(cnt,1)
  user_new = seg_sum(z[ucu_src]) / max(cnt,1),
  z = relu(LN(concat(h_user,user_ctx) @ W_conv + b_conv)) computed per user.

Device mechanics per 128-edge chunk: gpsimd ap_gather pulls rows
(feature-major) out of an SBUF-resident source-table segment, a PE
identity-matmul transposes them to edge-major, and a one-hot(dst) matmul
accumulates them into a PSUM [*, 512-dst-window] tile; counts ride the same
one-hot via a ones matmul. Host-side prep is layout-only (transposes,
edge permutation/padding, int16 index wrapping).
"""

import os
import sys
import types

import numpy as np
import ml_dtypes

import concourse.bacc as bacc
import concourse.mybir as mybir
import concourse.tile as tile
from concourse.bass_utils import run_bass_kernel_spmd

LAST_EXEC_NS = None


def _install_ntff_shim():
    """Register the axon NTFF profiling hook if absent (for HW timing)."""
    try:
        import antenv.axon_hooks  # noqa: F401

        return
    except ImportError:
        pass
    try:
        from trn_agent_boot.trn_boot import _ntff_profile_via_ctypes

        hook = _ntff_profile_via_ctypes("/opt/axon/libaxon_pjrt.so")
        mod = types.ModuleType("antenv.axon_hooks")
        mod.get_axon_ntff_profile_hook = lambda: hook
        sys.modules["antenv.axon_hooks"] = mod
    except Exception:
        pass


F32 = mybir.dt.float32
I16 = mybir.dt.int16
P = 128

IN_F = 128
OUT_F = 128
CONV_D = 64
LN_EPS = 1e-5
N_CORES = 8
N_SEG = 8
WIN = 512


def _pad_to(x, m):
    return ((x + m - 1) // m) * m


def prep_gather(src, dst, d_base, d_own, seg_size, n_win):
    """Edges with dst in [d_base, d_base+d_own), sorted by
    (src_segment, dst_window). Returns per-cell edge lists (src_rel int16,
    dst_rel float32 in [0, WIN))."""
    mask = (dst >= d_base) & (dst < d_base + d_own)
    s = src[mask].astype(np.int64)
    d = (dst[mask] - d_base).astype(np.int64)
    seg = s // seg_size
    w = d // WIN
    order = np.lexsort((d, w, seg))
    s, d, seg, w = s[order], d[order], seg[order], w[order]
    cells = {}
    key = seg * n_win + w
    bounds = np.searchsorted(key, np.arange(N_SEG * n_win + 1))
    for sg in range(N_SEG):
        for ww in range(n_win):
            k = sg * n_win + ww
            a, b = bounds[k], bounds[k + 1]
            cells[(sg, ww)] = (s[a:b] - sg * seg_size, d[a:b] - ww * WIN)
    return cells


def prep_eside(dst, feats, d_base, d_own, n_win):
    mask = (dst >= d_base) & (dst < d_base + d_own)
    d = (dst[mask] - d_base).astype(np.int64)
    rows = feats[mask]
    w = d // WIN
    order = np.lexsort((d, w))
    d, rows, w = d[order], rows[order], w[order]
    bounds = np.searchsorted(w, np.arange(n_win + 1))
    cells = {}
    for ww in range(n_win):
        a, b = bounds[ww], bounds[ww + 1]
        cells[ww] = (rows[a:b], d[a:b] - ww * WIN)
    return cells


def pack_gather_cells(cells_per_core, cc):
    """Pack per-core cell edge lists into unified padded slot arrays.
    Returns per-core (idx16 [128, S/16], dstc [128, S/128]) with S total
    slots = sum over cells of cc[cell]*128."""
    out = []
    order = sorted(cc.keys())
    for cells in cells_per_core:
        idx_parts, dst_parts = [], []
        for k in order:
            want = cc[k] * P
            sr, dr = cells[k]
            n = len(sr)
            sr2 = np.zeros(want, np.int16)
            dr2 = np.full(want, -1.0, np.float32)
            sr2[:n] = sr.astype(np.int16)
            dr2[:n] = dr.astype(np.float32)
            idx_parts.append(sr2)
            dst_parts.append(dr2)
        allsr = np.concatenate(idx_parts) if idx_parts else np.zeros(0, np.int16)
        alldr = np.concatenate(dst_parts) if dst_parts else np.zeros(0, np.float32)
        S = len(allsr)
        idx16 = np.tile(allsr.reshape(-1, 16).T, (8, 1)).copy() if S else \
            np.zeros((P, 0), np.int16)
        dstc = alldr.reshape(-1, P).T.copy() if S else np.zeros((P, 0), np.float32)
        out.append((idx16, dstc))
    return out


def pack_e_cells(cells_per_core, cc):
    out = []
    order = sorted(cc.keys())
    for cells in cells_per_core:
        row_parts, dst_parts = [], []
        for k in order:
            want = cc[k] * P
            rows, dr = cells[k]
            n = len(rows)
            r2 = np.zeros((want, CONV_D), np.float32)
            d2 = np.full(want, -1.0, np.float32)
            r2[:n] = rows
            d2[:n] = dr.astype(np.float32)
            row_parts.append(r2)
            dst_parts.append(d2)
        allr = np.concatenate(row_parts) if row_parts else \
            np.zeros((0, CONV_D), np.float32)
        alld = np.concatenate(dst_parts) if dst_parts else np.zeros(0, np.float32)
        dstc = alld.reshape(-1, P).T.copy() if len(alld) else \
            np.zeros((P, 0), np.float32)
        out.append((allr, dstc))
    return out


def build(n_user, d_own, seg_size, cc_com, cc_ucu, cc_pub, cc_e):
    nc = bacc.Bacc("TRN2", target_bir_lowering=False, debug=False,
                   num_devices=N_CORES)
    n_win = d_own // WIN
    useg = _pad_to(seg_size, P)
    ns_com = sum(cc_com.values()) * P
    ns_ucu = sum(cc_ucu.values()) * P
    ns_pub = sum(cc_pub.values()) * P
    ns_e = sum(cc_e.values()) * P

    def din(name, shape, dt=F32):
        return nc.dram_tensor(name, shape, dt, kind="ExternalInput")

    hT = din("hT", [P, N_SEG * useg])
    hT_bf = din("hT_bf", [P, N_SEG * useg], mybir.dt.bfloat16)
    ctxT = din("ctxT", [CONV_D, N_SEG * useg], mybir.dt.bfloat16)
    w_pub = din("w_pub", [IN_F, OUT_F])
    w_com = din("w_com", [IN_F, OUT_F])
    w_ecom = din("w_ecom", [CONV_D, OUT_F])
    w_conv = din("w_conv", [IN_F + CONV_D, OUT_F])
    biases = din("biases", [4, OUT_F])
    lnw = din("lnw", [2, OUT_F])
    com_idx = din("com_idx", [P, max(ns_com // 16, 1)], I16)
    com_dst = din("com_dst", [P, max(ns_com // P, 1)])
    ucu_idx = din("ucu_idx", [P, max(ns_ucu // 16, 1)], I16)
    ucu_dst = din("ucu_dst", [P, max(ns_ucu // P, 1)])
    pub_idx = din("pub_idx", [P, max(ns_pub // 16, 1)], I16)
    pub_dst = din("pub_dst", [P, max(ns_pub // P, 1)])
    e_rows = din("e_rows", [max(ns_e, P), CONV_D], mybir.dt.bfloat16)
    e_dst = din("e_dst", [P, max(ns_e // P, 1)])

    out = nc.dram_tensor("out", [3, d_own, OUT_F], F32, kind="ExternalOutput")
    zT_dram = nc.dram_tensor("zT_scratch", [P, N_SEG * useg], F32,
                             kind="Internal")

    with tile.TileContext(nc) as tc:
        with (
            tc.tile_pool(name="const", bufs=1) as cpool,
            tc.tile_pool(name="io", bufs=3) as iopool,
            tc.tile_pool(name="work", bufs=3) as wpool,
            tc.tile_pool(name="ps", bufs=1, space="PSUM") as pspool,
        ):
            # constants
            iota_i = cpool.tile([P, WIN], mybir.dt.int32)
            nc.gpsimd.iota(iota_i[:], pattern=[[1, WIN]], base=0,
                           channel_multiplier=0)
            iota_w = cpool.tile([P, WIN], F32)
            nc.vector.tensor_copy(iota_w[:], iota_i[:])
            ident = cpool.tile([P, P], F32)
            icol = cpool.tile([P, 1], F32)
            nc.vector.tensor_copy(icol[:], iota_i[:, :1])  # zeros col? no
            # identity via iota row == partition idx
            ic2 = cpool.tile([P, 1], mybir.dt.int32)
            nc.gpsimd.iota(ic2[:], pattern=[[1, 1]], base=0,
                           channel_multiplier=1)
            nc.vector.tensor_copy(icol[:], ic2[:])
            nc.vector.tensor_tensor(out=ident[:], in0=iota_w[:, :P],
                                    in1=icol[:].to_broadcast([P, P]),
                                    op=mybir.AluOpType.is_equal)
            ones_col = cpool.tile([P, 1], F32)
            nc.vector.memset(ones_col[:], 1.0)

            wS = cpool.tile([IN_F, OUT_F], F32, tag="t_wpub")
            nc.sync.dma_start(wS[:], w_pub[:])
            wC7 = cpool.tile([IN_F, OUT_F], F32, tag="t_wcom7")
            nc.sync.dma_start(wC7[:], w_com[:])
            nc.vector.tensor_scalar_mul(wC7[:], wC7[:], 0.7)
            wE3 = cpool.tile([CONV_D, OUT_F], F32, tag="t_wecom3")
            nc.sync.dma_start(wE3[:], w_ecom[:])
            nc.vector.tensor_scalar_mul(wE3[:], wE3[:], 0.3)
            wV1 = cpool.tile([IN_F, OUT_F], F32, tag="t_wconv1")
            nc.sync.dma_start(wV1[:], w_conv[:IN_F, :])
            wV2 = cpool.tile([CONV_D, OUT_F], F32, tag="t_wconv2")
            nc.sync.dma_start(wV2[:], w_conv[IN_F:, :])
            b_pub_sb = cpool.tile([1, OUT_F], F32, tag="t_bp")
            nc.sync.dma_start(b_pub_sb[:], biases[0:1, :])
            b_com_sb = cpool.tile([1, OUT_F], F32, tag="t_bc")
            nc.sync.dma_start(b_com_sb[:], biases[1:2, :])
            b_ecom_sb = cpool.tile([1, OUT_F], F32, tag="t_be")
            nc.sync.dma_start(b_ecom_sb[:], biases[2:3, :])
            b_conv_sb = cpool.tile([1, OUT_F], F32, tag="t_bv")
            nc.sync.dma_start(b_conv_sb[:], biases[3:4, :])
            bmix = cpool.tile([1, OUT_F], F32, tag="t_bmix")
            nc.vector.tensor_scalar_mul(bmix[:], b_com_sb[:], 0.7)
            tmpb = cpool.tile([1, OUT_F], F32, tag="t_tmpb")
            nc.vector.tensor_scalar_mul(tmpb[:], b_ecom_sb[:], 0.3)
            nc.vector.tensor_add(bmix[:], bmix[:], tmpb[:])
            g_sb = cpool.tile([1, OUT_F], F32, tag="t_g")
            nc.sync.dma_start(g_sb[:], lnw[0:1, :])
            lb_sb = cpool.tile([1, OUT_F], F32, tag="t_lb")
            nc.sync.dma_start(lb_sb[:], lnw[1:2, :])
            wC7b = cpool.tile([IN_F, OUT_F], mybir.dt.bfloat16,
                              tag="t_wcom7b")
            nc.vector.tensor_copy(wC7b[:], wC7[:])
            wE3b = cpool.tile([CONV_D, OUT_F], mybir.dt.bfloat16,
                              tag="t_wecom3b")
            nc.vector.tensor_copy(wE3b[:], wE3[:])
            bmixb = cpool.tile([1, OUT_F], mybir.dt.bfloat16, tag="t_bmixb")
            nc.vector.tensor_copy(bmixb[:], bmix[:])
            bpubb = cpool.tile([1, OUT_F], mybir.dt.bfloat16, tag="t_bpubb")
            nc.vector.tensor_copy(bpubb[:], b_pub_sb[:])
            ident_b = cpool.tile([P, P], mybir.dt.bfloat16, tag="t_identb")
            nc.vector.tensor_copy(ident_b[:], ident[:])
            ones_col_b = cpool.tile([P, 1], mybir.dt.bfloat16, tag="t_onescb")
            nc.vector.tensor_copy(ones_col_b[:], ones_col[:])
            wV1b = cpool.tile([IN_F, OUT_F], mybir.dt.bfloat16, tag="t_wv1b")
            nc.vector.tensor_copy(wV1b[:], wV1[:])
            wV2b = cpool.tile([CONV_D, OUT_F], mybir.dt.bfloat16, tag="t_wv2b")
            nc.vector.tensor_copy(wV2b[:], wV2[:])
            ones_row = cpool.tile([1, P], F32, tag="t_onesrow")
            nc.vector.memset(ones_row[:], 1.0)

            def replicate(row_ap, tag):
                psr = pspool.tile([P, OUT_F], F32, tag="t1")
                nc.tensor.matmul(psr[:], lhsT=ones_row[:], rhs=row_ap,
                                 start=True, stop=True)
                t = cpool.tile([P, OUT_F], F32, tag=tag)
                nc.scalar.copy(t[:], psr[:])
                return t

            bconv_rep = replicate(b_conv_sb[:], "t_bconvrep")
            g_rep = replicate(g_sb[:], "t_grep")
            lb_rep = replicate(lb_sb[:], "t_lbrep")

            # ---------- phase Z ----------
            n_uch = N_SEG * useg // P
            for uc in range(n_uch):
                hT_c = iopool.tile([P, P], mybir.dt.bfloat16, tag="hTc")
                nc.sync.dma_start(hT_c[:], hT_bf[:, uc * P : (uc + 1) * P])
                cT_c = iopool.tile([CONV_D, P], mybir.dt.bfloat16, tag="cTc")
                nc.sync.dma_start(cT_c[:], ctxT[:, uc * P : (uc + 1) * P])
                zps = pspool.tile([P, OUT_F], F32, tag="t1")
                nc.tensor.matmul(zps[:], lhsT=hT_c[:], rhs=wV1b[:],
                                 start=True, stop=False)
                nc.tensor.matmul(zps[:], lhsT=cT_c[:], rhs=wV2b[:],
                                 start=False, stop=True)
                zr = wpool.tile([P, OUT_F], F32, tag="zr")
                nc.vector.tensor_tensor(out=zr[:], in0=zps[:],
                                        in1=bconv_rep[:],
                                        op=mybir.AluOpType.add)
                mu = wpool.tile([P, 1], F32, tag="mu")
                nc.vector.reduce_sum(mu[:], zr[:], axis=mybir.AxisListType.X)
                nc.vector.tensor_scalar_mul(mu[:], mu[:], 1.0 / OUT_F)
                nc.vector.tensor_scalar(out=zr[:], in0=zr[:], scalar1=mu[:],
                                        scalar2=None,
                                        op0=mybir.AluOpType.subtract)
                sq = wpool.tile([P, OUT_F], F32, tag="sq")
                nc.vector.tensor_tensor(out=sq[:], in0=zr[:], in1=zr[:],
                                        op=mybir.AluOpType.mult)
                var = wpool.tile([P, 1], F32, tag="var")
                nc.vector.reduce_sum(var[:], sq[:], axis=mybir.AxisListType.X)
                nc.vector.tensor_scalar(out=var[:], in0=var[:],
                                        scalar1=1.0 / OUT_F, scalar2=LN_EPS,
                                        op0=mybir.AluOpType.mult,
                                        op1=mybir.AluOpType.add)
                sd = wpool.tile([P, 1], F32, tag="sd")
                nc.scalar.activation(sd[:], var[:],
                                     mybir.ActivationFunctionType.Sqrt)
                rs = wpool.tile([P, 1], F32, tag="rs")
                nc.vector.reciprocal(rs[:], sd[:])
                nc.vector.tensor_scalar(out=zr[:], in0=zr[:], scalar1=rs[:],
                                        scalar2=None,
                                        op0=mybir.AluOpType.mult)
                nc.vector.tensor_tensor(out=zr[:], in0=zr[:],
                                        in1=g_rep[:],
                                        op=mybir.AluOpType.mult)
                nc.vector.tensor_tensor(out=zr[:], in0=zr[:],
                                        in1=lb_rep[:],
                                        op=mybir.AluOpType.add)
                nc.vector.tensor_scalar_max(zr[:], zr[:], 0.0)
                zTps = pspool.tile([P, P], F32, tag="t2")
                nc.tensor.transpose(zTps[:], zr[:], ident[:])
                zTsb = wpool.tile([P, P], F32, tag="zTsb")
                nc.scalar.copy(zTsb[:], zTps[:])
                nc.sync.dma_start(zT_dram[:, uc * P : (uc + 1) * P], zTsb[:])

            # ---------- generic gather sweep ----------
            def gather_sweep(table, idx_t, dst_t, cc, accT, cnt_acc, tag,
                             ns, segpool, idxpool):
                if ns == 0:
                    return
                seg_nslots = [sum(cc[(sg, ww)] for ww in range(n_win)) * P
                              for sg in range(N_SEG)]
                max_segn = max(max(seg_nslots), P)
                slot0 = 0
                for sg in range(N_SEG):
                    if seg_nslots[sg] == 0:
                        continue
                    segn = seg_nslots[sg]
                    seg_start = slot0
                    idx_sb = idxpool.tile([P, max_segn // 16], I16,
                                          tag="segidx")
                    nc.sync.dma_start(
                        idx_sb[:, : segn // 16],
                        idx_t[:, seg_start // 16 : (seg_start + segn) // 16])
                    dst_sb = idxpool.tile([P, max_segn // P], F32,
                                          tag="segdst")
                    nc.sync.dma_start(
                        dst_sb[:, : segn // P],
                        dst_t[:, seg_start // P : (seg_start + segn) // P])
                    seg_sb = segpool.tile([P, useg], F32, tag="segtab")
                    nc.sync.dma_start(seg_sb[:],
                                      table[:, sg * useg : (sg + 1) * useg])
                    for ww in range(n_win):
                        nch = cc[(sg, ww)]
                        if nch == 0:
                            continue
                        ps_acc = pspool.tile([P, WIN], F32, tag="psacc")
                        if cnt_acc is not None:
                            ps_cnt = pspool.tile([1, WIN], F32, tag="pscnt")
                        else:
                            ps_cnt = None
                        # one gather for the whole cell (<=16 chunks each)
                        for c0 in range(0, nch, 8):
                            cn = min(8, nch - c0)
                            g = wpool.tile([P, 8 * P], F32, tag="gg")
                            s0 = slot0 - seg_start + c0 * P
                            nc.gpsimd.ap_gather(
                                out_ap=g[:, : cn * P], in_ap=seg_sb[:],
                                idxs_ap=idx_sb[:, s0 // 16 :
                                               (s0 + cn * P) // 16],
                                channels=P, num_elems=useg, d=1,
                                num_idxs=cn * P)
                            for c in range(cn):
                                cc_abs = c0 + c
                                rps = pspool.tile([P, P], F32, tag="t2")
                                nc.tensor.transpose(
                                    rps[:], g[:, c * P : (c + 1) * P],
                                    ident[:])
                                rows = wpool.tile([P, P], F32, tag="gr")
                                nc.scalar.copy(rows[:], rps[:])
                                oh = wpool.tile([P, WIN], F32, tag="go")
                                col = (slot0 - seg_start) // P + cc_abs
                                nc.vector.tensor_tensor(
                                    out=oh[:], in0=iota_w[:],
                                    in1=dst_sb[:, col : col + 1]
                                    .to_broadcast([P, WIN]),
                                    op=mybir.AluOpType.is_equal)
                                if cnt_acc is not None:
                                    oh_b = wpool.tile(
                                        [P, WIN], mybir.dt.bfloat16,
                                        tag="gob")
                                    nc.vector.tensor_tensor(
                                        out=oh_b[:], in0=iota_w[:],
                                        in1=dst_sb[:, col : col + 1]
                                        .to_broadcast([P, WIN]),
                                        op=mybir.AluOpType.is_equal)
                                nc.tensor.matmul(
                                    ps_acc[:], lhsT=rows[:], rhs=oh[:],
                                    start=(cc_abs == 0),
                                    stop=(cc_abs == nch - 1))
                                if ps_cnt is not None:
                                    nc.tensor.matmul(
                                        ps_cnt[:], lhsT=ones_col_b[:, :1],
                                        rhs=oh_b[:], start=(cc_abs == 0),
                                        stop=(cc_abs == nch - 1))
                        nc.vector.tensor_add(
                            accT[:, ww * WIN : (ww + 1) * WIN],
                            accT[:, ww * WIN : (ww + 1) * WIN], ps_acc[:])
                        if ps_cnt is not None:
                            nc.vector.tensor_add(
                                cnt_acc[:, ww * WIN : (ww + 1) * WIN],
                                cnt_acc[:, ww * WIN : (ww + 1) * WIN],
                                ps_cnt[:])
                        slot0 += nch * P

            def finalize(ww, terms, cnt_row, out_idx, opool):
                """terms: list of (lhsT_ap, rhs_ap); out = (sum terms) /
                max(cnt,1) written to out[out_idx, ww*WIN: ...]."""
                for j in range(WIN // P):
                    sl = slice(ww * WIN + j * P, ww * WIN + (j + 1) * P)
                    pso = pspool.tile([P, OUT_F], F32, tag="t1")
                    for i, (lh, rh) in enumerate(terms):
                        nc.tensor.matmul(pso[:], lhsT=lh[:, sl], rhs=rh,
                                         start=(i == 0),
                                         stop=(i == len(terms) - 1))
                    crow = opool.tile([1, P], F32, tag="crow")
                    nc.vector.tensor_copy(crow[:], cnt_row[:, sl])
                    cps = pspool.tile([P, 1], F32, tag="t2")
                    nc.tensor.transpose(cps[:], crow[:], ident[:1, :1])
                    cc_ = opool.tile([P, 1], F32, tag="ccl")
                    nc.vector.tensor_scalar_max(cc_[:], cps[:], 1.0)
                    rec = opool.tile([P, 1], F32, tag="rec")
                    nc.vector.reciprocal(rec[:], cc_[:])
                    osb = opool.tile([P, OUT_F], F32, tag="osb")
                    nc.vector.tensor_scalar(out=osb[:], in0=pso[:],
                                            scalar1=rec[:], scalar2=None,
                                            op0=mybir.AluOpType.mult)
                    nc.sync.dma_start(out[out_idx, sl, :], osb[:])

            # ========== relation: com (+ e-side) ==========
            with (
                tc.tile_pool(name="seg1", bufs=1) as segpool,
                tc.tile_pool(name="idx1", bufs=1) as idxpool,
                tc.tile_pool(name="acc1", bufs=1) as accpool,
            ):
                accw = n_win * WIN
                S_h = accpool.tile([P, accw], mybir.dt.bfloat16, tag="Sh")
                nc.vector.memset(S_h[:], 0.0)
                S_e = accpool.tile([CONV_D, accw], mybir.dt.bfloat16, tag="Se")
                nc.vector.memset(S_e[:], 0.0)
                c_e = accpool.tile([1, accw], mybir.dt.bfloat16, tag="ce")
                nc.vector.memset(c_e[:], 0.0)
                gather_sweep(hT, com_idx, com_dst, cc_com, S_h, None, "cm",
                             ns_com, segpool, idxpool)
                # e-side
                if ns_e:
                    ed_sb = idxpool.tile([P, ns_e // P], F32, tag="edsb")
                    nc.sync.dma_start(ed_sb[:], e_dst[:, : ns_e // P])
                    slot0 = 0
                    for ww in range(n_win):
                        nch = cc_e[ww]
                        if nch == 0:
                            continue
                        ps_e = pspool.tile([CONV_D, WIN], F32, tag="pse")
                        ps_ec = pspool.tile([1, WIN], F32, tag="psec")
                        for c in range(nch):
                            s0 = slot0 + c * P
                            er = wpool.tile([P, CONV_D], mybir.dt.bfloat16, tag="er")
                            nc.sync.dma_start(er[:], e_rows[s0 : s0 + P, :])
                            oh = wpool.tile([P, WIN], mybir.dt.bfloat16, tag="eoh")
                            nc.vector.tensor_tensor(
                                out=oh[:], in0=iota_w[:],
                                in1=ed_sb[:, s0 // P : s0 // P + 1]
                                .to_broadcast([P, WIN]),
                                op=mybir.AluOpType.is_equal)
                            nc.tensor.matmul(ps_e[:], lhsT=er[:],
                                             rhs=oh[:], start=(c == 0),
                                             stop=(c == nch - 1))
                            nc.tensor.matmul(ps_ec[:],
                                             lhsT=ones_col_b[:, :1], rhs=oh[:],
                                             start=(c == 0),
                                             stop=(c == nch - 1))
                        nc.vector.tensor_add(
                            S_e[:, ww * WIN : (ww + 1) * WIN],
                            S_e[:, ww * WIN : (ww + 1) * WIN], ps_e[:])
                        nc.vector.tensor_add(
                            c_e[:, ww * WIN : (ww + 1) * WIN],
                            c_e[:, ww * WIN : (ww + 1) * WIN], ps_ec[:])
                        slot0 += nch * P
                for ww in range(n_win):
                    finalize(ww,
                             [(S_h, wC7b[:]), (S_e, wE3b[:]),
                              (c_e, bmixb[:])],
                             c_e, 1, wpool)

            # ========== relation: pub ==========
            with (
                tc.tile_pool(name="seg2", bufs=1) as segpool,
                tc.tile_pool(name="idx2", bufs=1) as idxpool,
                tc.tile_pool(name="acc2", bufs=1) as accpool,
            ):
                accw = n_win * WIN
                S_p = accpool.tile([P, accw], F32, tag="Sp")
                nc.vector.memset(S_p[:], 0.0)
                c_p = accpool.tile([1, accw], mybir.dt.bfloat16, tag="cp")
                nc.vector.memset(c_p[:], 0.0)
                gather_sweep(hT, pub_idx, pub_dst, cc_pub, S_p, c_p, "pb",
                             ns_pub, segpool, idxpool)
                for ww in range(n_win):
                    finalize(ww, [(S_p, wS[:]), (c_p, bpubb[:])],
                             c_p, 0, wpool)

            # ========== relation: ucu ==========
            with (
                tc.tile_pool(name="seg3", bufs=1) as segpool,
                tc.tile_pool(name="idx3", bufs=1) as idxpool,
                tc.tile_pool(name="acc3", bufs=1) as accpool,
            ):
                accw = n_win * WIN
                S_z = accpool.tile([P, accw], F32, tag="Sz")
                nc.vector.memset(S_z[:], 0.0)
                c_u = accpool.tile([1, accw], mybir.dt.bfloat16, tag="cu")
                nc.vector.memset(c_u[:], 0.0)
                gather_sweep(zT_dram, ucu_idx, ucu_dst, cc_ucu, S_z, c_u,
                             "uc", ns_ucu, segpool, idxpool)
                for ww in range(n_win):
                    finalize(ww, [(S_z, ident[:])], c_u, 2, wpool)

    nc.compile()
    return nc


def kernel(h_user, h_post, user_ctx, e_comment, pub_src, pub_dst, com_src,
           com_dst, ucu_src, ucu_dst, W_pub, b_pub, W_com, b_com, W_conv,
           b_conv, ln_g, ln_b, W_ecom, b_ecom):
    h_user = np.asarray(h_user, np.float32)
    user_ctx = np.asarray(user_ctx, np.float32)
    e_comment = np.asarray(e_comment, np.float32)
    n_user = h_user.shape[0]
    n_post = np.asarray(h_post).shape[0]
    n_out = max(n_user, n_post)
    d_own = _pad_to((n_out + N_CORES - 1) // N_CORES, WIN)
    n_win = d_own // WIN
    seg_size = (n_user + N_SEG - 1) // N_SEG
    useg = _pad_to(seg_size, P)

    hT = np.zeros((P, N_SEG * useg), np.float32)
    ctxT = np.zeros((CONV_D, N_SEG * useg), ml_dtypes.bfloat16)
    hts = np.ascontiguousarray(h_user.T)
    cts = np.ascontiguousarray(user_ctx.T.astype(ml_dtypes.bfloat16))
    for sg in range(N_SEG):
        a, b = sg * seg_size, min((sg + 1) * seg_size, n_user)
        hT[:, sg * useg : sg * useg + (b - a)] = hts[:, a:b]
        ctxT[:, sg * useg : sg * useg + (b - a)] = cts[:, a:b]

    arr = lambda x: np.asarray(x)
    cells_com, cells_ucu, cells_pub, cells_e = [], [], [], []
    for c in range(N_CORES):
        d_base = c * d_own
        cells_com.append(prep_gather(arr(com_src), arr(com_dst), d_base,
                                     d_own, seg_size, n_win))
        cells_ucu.append(prep_gather(arr(ucu_src), arr(ucu_dst), d_base,
                                     d_own, seg_size, n_win))
        cells_pub.append(prep_gather(arr(pub_src), arr(pub_dst), d_base,
                                     d_own, seg_size, n_win))
        cells_e.append(prep_eside(arr(com_dst), e_comment, d_base, d_own,
                                  n_win))

    def unify(cells_list, keys):
        return {k: max((len(cl[k][0]) + P - 1) // P for cl in cells_list)
                for k in keys}

    gkeys = [(sg, ww) for sg in range(N_SEG) for ww in range(n_win)]
    cc_com = unify(cells_com, gkeys)
    cc_ucu = unify(cells_ucu, gkeys)
    cc_pub = unify(cells_pub, gkeys)
    cc_e = {ww: max((len(cl[ww][0]) + P - 1) // P for cl in cells_e)
            for ww in range(n_win)}

    nc = build(n_user, d_own, seg_size, cc_com, cc_ucu, cc_pub, cc_e)

    packed_com = pack_gather_cells(cells_com, cc_com)
    packed_ucu = pack_gather_cells(cells_ucu, cc_ucu)
    packed_pub = pack_gather_cells(cells_pub, cc_pub)
    packed_e = pack_e_cells(cells_e, cc_e)

    biases = np.stack([arr(b_pub), arr(b_com), arr(b_ecom),
                       arr(b_conv)]).astype(np.float32)
    lnw = np.stack([arr(ln_g), arr(ln_b)]).astype(np.float32)
    ns_e = sum(cc_e.values()) * P

    in_maps = []
    for c in range(N_CORES):
        ci, cd = packed_com[c]
        ui, ud = packed_ucu[c]
        pi, pd = packed_pub[c]
        er, ed = packed_e[c]
        er_pad = np.zeros((max(ns_e, P), CONV_D), ml_dtypes.bfloat16)
        er_pad[: len(er)] = er
        m = {
            "hT": hT, "hT_bf": hT.astype(ml_dtypes.bfloat16),
            "ctxT": ctxT,
            "w_pub": arr(W_pub).astype(np.float32),
            "w_com": arr(W_com).astype(np.float32),
            "w_ecom": arr(W_ecom).astype(np.float32),
            "w_conv": arr(W_conv).astype(np.float32),
            "biases": biases, "lnw": lnw,
            "com_idx": _fit(ci, np.int16), "com_dst": _fit(cd, np.float32),
            "ucu_idx": _fit(ui, np.int16), "ucu_dst": _fit(ud, np.float32),
            "pub_idx": _fit(pi, np.int16), "pub_dst": _fit(pd, np.float32),
            "e_rows": er_pad, "e_dst": _fit(ed, np.float32),
        }
        in_maps.append(m)

    trace = bool(os.environ.get("KERNEL_TRACE"))
    if trace:
        _install_ntff_shim()
    res = run_bass_kernel_spmd(nc, in_maps, list(range(N_CORES)),
                               trace=trace)
    global LAST_EXEC_NS
    LAST_EXEC_NS = getattr(res, "exec_time_ns", None)
    outs = [r["out"] for r in res.results]
    full = np.concatenate(outs, axis=1)
    return full[:, :n_post, :].astype(np.float32)


def _fit(a, dt):
    if a.shape[1] == 0:
        return np.zeros((a.shape[0], 1), dt)
    return np.ascontiguousarray(a.astype(dt))

